# revision 7
# baseline (speedup 1.0000x reference)
"""Cluster (retrieval_knn) kernel for 8 TRN2 NeuronCores.

Device (SPMD, batch-sharded 4096 rows/core):
  - cos = feature @ l2norm(centroids).T via fp16 hi/lo 3-term split
    (fh@ch + fh@cl + fl@ch, fp32 PSUM accumulate; ~2^-22 rel error)
  - label = argmax_c cos, rawmax = max_c cos (DVE InstMax/InstMaxIndex)
  - cos_f = rawmax / ||feature_row||  (row norm precomputed on host)
  - scores = 1/sum(exp(pred - rowmax))  (ACT exp with accum, host rowmax bias)
Host: transposed fp16 hi/lo feature split, per-class stats + weight,
sequential top-N scatter-insert (vectorized over classes), selected_label.
"""
import sys

for _p in ("/opt/trn_rl_repo", "/root/.axon_site/_ro/trn_rl_repo"):
    if _p not in sys.path:
        sys.path.insert(0, _p)

import numpy as np

B, C, D, K = 32768, 1000, 256, 16
NCORES = 8
BS = B // NCORES          # 4096 samples per core
P = 128                   # partitions
NT = BS // P              # 32 tiles per core
ULB_LEN = 50000
EPS = 1e-8

_BUILT = None


def _build():
    import concourse.bacc as bacc
    import concourse.mybir as mybir
    from concourse.tile import TileContext
    from concourse.masks import make_identity

    fp32 = mybir.dt.float32
    fp16 = mybir.dt.float16
    nc = bacc.Bacc("TRN2", target_bir_lowering=False, debug=False)

    fht_d = nc.dram_tensor("fht", [P, 2, BS], fp16, kind="ExternalInput")
    flt_d = nc.dram_tensor("flt", [P, 2, BS], fp16, kind="ExternalInput")
    pred_d = nc.dram_tensor("pred", [BS, C], fp32, kind="ExternalInput")
    cent_d = nc.dram_tensor("cent", [C, D], fp32, kind="ExternalInput")
    nmax_d = nc.dram_tensor("nmax", [P, NT], fp32, kind="ExternalInput")

    maxa_d = nc.dram_tensor("maxa", [P, NT], fp32, kind="ExternalOutput")
    maxb_d = nc.dram_tensor("maxb", [P, NT], fp32, kind="ExternalOutput")
    idxa_d = nc.dram_tensor("idxa", [P, NT], fp32, kind="ExternalOutput")
    idxb_d = nc.dram_tensor("idxb", [P, NT], fp32, kind="ExternalOutput")
    score_d = nc.dram_tensor("score", [P, NT], fp32, kind="ExternalOutput")

    n_ct = (C + P - 1) // P  # 8 centroid tiles, last has 104 rows

    with TileContext(nc) as tc:
        with tc.tile_pool(name="sb", bufs=1) as sb, \
             tc.tile_pool(name="ps", bufs=1, space="PSUM") as ps:

            ident = sb.tile([P, P], fp32)
            make_identity(nc, ident[:])

            # persistent transposed fp16 feature splits
            fht = sb.tile([P, 2, BS], fp16)
            flt = sb.tile([P, 2, BS], fp16)
            CH = BS // 4
            for ci in range(4):
                cs = slice(ci * CH, (ci + 1) * CH)
                nc.sync.dma_start(fht[:, :, cs], fht_d[:, :, cs])
                nc.sync.dma_start(flt[:, :, cs], flt_d[:, :, cs])

            # ---- setup: normalize + transpose centroids -> chT/clT [128,2,1000] fp16
            chT = sb.tile([P, 2, C], fp16)
            clT = sb.tile([P, 2, C], fp16)
            for ct in range(n_ct):
                r0 = ct * P
                rows = min(P, C - r0)
                cl_ = sb.tile([P, D], fp32, tag="centload", bufs=2)
                nc.sync.dma_start(cl_[:rows], cent_d[r0:r0 + rows, :])
                csq = sb.tile([P, D], fp32, tag="csq", bufs=2)
                css = sb.tile([P, 1], fp32, tag="css", bufs=2)
                nc.scalar.activation(csq[:rows], cl_[:rows],
                                     mybir.ActivationFunctionType.Square,
                                     accum_out=css[:rows])
                cnrm = sb.tile([P, 1], fp32, tag="cnrm", bufs=2)
                nc.scalar.sqrt(cnrm[:rows], css[:rows])
                nc.vector.tensor_scalar_max(cnrm[:rows], cnrm[:rows], EPS)
                crcp = sb.tile([P, 1], fp32, tag="crcp", bufs=2)
                nc.vector.reciprocal(crcp[:rows], cnrm[:rows])
                cnormed = sb.tile([P, D], fp32, tag="cnormed", bufs=2)
                if rows < P:
                    nc.vector.memset(cnormed[:], 0.0)
                nc.scalar.mul(cnormed[:rows], cl_[:rows], crcp[:rows])
                for k in range(2):
                    cps = ps.tile([P, P], fp32, space="PSUM", tag="tps", bufs=2)
                    nc.tensor.transpose(out=cps[:], in_=cnormed[:, k * P:(k + 1) * P],
                                        identity=ident[:])
                    nc.scalar.copy(chT[:, k, r0:r0 + rows], cps[:, :rows])
                    # clT = cps - chT (exact fp16 residual)
                    cres = sb.tile([P, P], fp32, tag="cres", bufs=2)
                    nc.vector.tensor_tensor(out=cres[:, :rows], in0=cps[:, :rows],
                                            in1=chT[:, k, r0:r0 + rows],
                                            op=mybir.AluOpType.subtract)
                    nc.vector.tensor_copy(clT[:, k, r0:r0 + rows], cres[:, :rows])

            nmax = sb.tile([P, NT], fp32)
            nc.sync.dma_start(nmax[:], nmax_d[:, :])

            maxa_cols = sb.tile([P, NT], fp32)
            maxb_cols = sb.tile([P, NT], fp32)
            idxa_cols = sb.tile([P, NT], fp32)
            idxb_cols = sb.tile([P, NT], fp32)
            ssum_cols = sb.tile([P, NT], fp32)
            sc_cols = sb.tile([P, NT], fp32)

            # ---- main loop over 32 sample tiles
            for t in range(NT):
                s0 = t * P
                pred_t = sb.tile([P, C], fp32, tag="pred", bufs=6)
                nc.sync.dma_start(pred_t[:], pred_d[s0:s0 + P, :])

                # scores = 1 / sum(exp(pred - max))
                esc = sb.tile([P, C], fp32, tag="esc", bufs=3)
                nc.scalar.activation(esc[:], pred_t[:],
                                     mybir.ActivationFunctionType.Exp,
                                     bias=nmax[:, t:t + 1],
                                     accum_out=ssum_cols[:, t:t + 1])

                # cos = fh@ch + fh@cl + fl@ch  (fp16, fp32 accum)
                sl = slice(s0, s0 + P)
                for h, (c0, c1) in enumerate(((0, 512), (512, C))):
                    cos_ps = ps.tile([P, c1 - c0], fp32, space="PSUM",
                                     tag=f"cos{h}", bufs=3)
                    first = True
                    for (lt, rt) in ((fht, chT), (fht, clT), (flt, chT)):
                        for k in range(2):
                            last = lt is flt and k == 1
                            nc.tensor.matmul(cos_ps[:, :], lt[:, k, sl],
                                             rt[:, k, c0:c1], start=first, stop=last)
                            first = False
                    max8 = sb.tile([P, 8], fp32, tag="max8", bufs=4)
                    idx8 = sb.tile([P, 8], mybir.dt.uint32, tag="idx8", bufs=4)
                    nc.vector.max(out=max8[:], in_=cos_ps[:])
                    nc.vector.max_index(out=idx8[:], in_max=max8[:],
                                        in_values=cos_ps[:])
                    mc = maxa_cols if h == 0 else maxb_cols
                    ic = idxa_cols if h == 0 else idxb_cols
                    nc.vector.tensor_copy(mc[:, t:t + 1], max8[:, 0:1])
                    nc.vector.tensor_copy(ic[:, t:t + 1], idx8[:, 0:1])

            nc.vector.reciprocal(sc_cols[:], ssum_cols[:])
            nc.sync.dma_start(maxa_d[:, :], maxa_cols[:])
            nc.sync.dma_start(maxb_d[:, :], maxb_cols[:])
            nc.sync.dma_start(idxa_d[:, :], idxa_cols[:])
            nc.sync.dma_start(idxb_d[:, :], idxb_cols[:])
            nc.sync.dma_start(score_d[:, :], sc_cols[:])

    nc.compile()
    return nc


def _host_prep(feature, pred):
    """Per-core input maps: fp16 hi/lo transposed feature, -rowmax(pred), 1/||f||."""
    prep = []
    for c in range(NCORES):
        sl = slice(c * BS, (c + 1) * BS)
        f = feature[sl]                                   # [BS, D] f32
        fh = f.astype(np.float16)
        fl = (f - fh.astype(np.float32)).astype(np.float16)
        # [BS, D] -> [P, 2, BS] with [p, k, i] = x[i, 128k+p]
        fht = np.ascontiguousarray(fh.T.reshape(2, P, BS).transpose(1, 0, 2))
        flt = np.ascontiguousarray(fl.T.reshape(2, P, BS).transpose(1, 0, 2))
        pshard = np.ascontiguousarray(pred[sl])
        negmax = (-pshard.max(axis=1)).astype(np.float32)
        nrm = np.sqrt((f.astype(np.float32) ** 2).sum(axis=1, dtype=np.float32))
        finv = (1.0 / np.maximum(nrm, np.float32(EPS))).astype(np.float32)
        prep.append(({
            "fht": fht, "flt": flt, "pred": pshard,
            "nmax": np.ascontiguousarray(negmax.reshape(NT, P).T),
        }, finv))
    return prep


def _run_device(feature, pred, centroids):
    global _BUILT
    from concourse.bass_utils import run_bass_kernel_spmd

    if _BUILT is None:
        _BUILT = _build()
    nc = _BUILT

    feature = np.ascontiguousarray(np.asarray(feature, dtype=np.float32))
    pred = np.ascontiguousarray(np.asarray(pred, dtype=np.float32))
    cent = np.ascontiguousarray(np.asarray(centroids, dtype=np.float32))

    prep = _host_prep(feature, pred)
    in_maps = []
    finvs = []
    for m, finv in prep:
        m["cent"] = cent
        in_maps.append(m)
        finvs.append(finv)

    res = run_bass_kernel_spmd(nc, in_maps, core_ids=list(range(NCORES)))

    label = np.empty(B, dtype=np.float32)
    cosf = np.empty(B, dtype=np.float32)
    score = np.empty(B, dtype=np.float32)
    for c in range(NCORES):
        r = res.results[c]
        sl = slice(c * BS, (c + 1) * BS)
        ma = r["maxa"].T.reshape(-1)
        mb = r["maxb"].T.reshape(-1)
        ia = r["idxa"].T.reshape(-1)
        ib = r["idxb"].T.reshape(-1)
        b_wins = mb > ma                      # strict: ties -> first half
        label[sl] = np.where(b_wins, ib + 512.0, ia).astype(np.float32)
        cosf[sl] = np.maximum(ma, mb) * finvs[c]
        score[sl] = r["score"].T.reshape(-1)
    return label, cosf, score, res


def kernel(feature, pred, centroids, pred_top_N, unlabeled_index):
    feature = np.asarray(feature)
    pred = np.asarray(pred)
    centroids = np.asarray(centroids)
    pred_top_N = np.asarray(pred_top_N, dtype=np.float32)
    unlabeled_index = np.asarray(unlabeled_index)

    label_f, cos_f, scores, _ = _run_device(feature, pred, centroids)
    lab = label_f.astype(np.int64)

    # per-class stats (mirrors reference segment sums + var formula, f32)
    cnt = np.bincount(lab, minlength=C).astype(np.float32)
    s1 = np.bincount(lab, weights=cos_f.astype(np.float64), minlength=C).astype(np.float32)
    s2 = np.bincount(lab, weights=(cos_f.astype(np.float64) ** 2), minlength=C).astype(np.float32)
    mean = s1 / np.maximum(cnt, np.float32(1.0))
    var = (s2 - cnt * mean * mean) / np.maximum(cnt - np.float32(1.0), np.float32(1.0))
    std = np.sqrt(np.maximum(var, np.float32(0.0)) + np.float32(1e-12))

    m_i = mean[lab]
    s_i = std[lab]
    inv_sqrt_2pi = np.float32(1.0 / np.sqrt(2.0 * np.pi))
    z = (cos_f - m_i) / s_i
    pdf = np.exp(np.float32(-0.5) * z * z) * inv_sqrt_2pi / s_i
    weight = np.where(cos_f < m_i, pdf, np.float32(1.0)).astype(np.float32)

    # sequential scatter-insert, vectorized over classes (exact semantics):
    # round r processes the r-th sample of each class in batch order.
    topN = pred_top_N.copy()
    fidx = np.full((C, K), -1, dtype=np.int32)
    order = np.argsort(lab, kind="stable")
    lab_o = lab[order]
    rank = np.arange(B) - np.searchsorted(lab_o, lab_o, side="left")
    counts = np.bincount(lab, minlength=C)
    maxn = int(counts.max()) if B else 0
    sc_mat = np.full((C, maxn), -np.inf, dtype=np.float32)
    ui_mat = np.zeros((C, maxn), dtype=np.int64)
    sc_mat[lab_o, rank] = scores[order]
    ui_mat[lab_o, rank] = unlabeled_index.astype(np.int64)[order]
    for r in range(maxn):
        s = sc_mat[:, r]
        mi = np.argmin(topN, axis=1)
        mv = topN[np.arange(C), mi]
        do = s > mv
        rows = np.nonzero(do)[0]
        topN[rows, mi[rows]] = s[rows]
        fidx[rows, mi[rows]] = ui_mat[rows, r].astype(np.int32)

    selected_label = np.full(ULB_LEN, -1, dtype=np.int32)
    selected_label[unlabeled_index.astype(np.int64)] = lab.astype(np.int32)

    return (label_f.astype(np.float32), weight, topN.astype(np.float32),
            fidx, selected_label)
